# revision 13
# baseline (speedup 1.0000x reference)
"""Causal self-attention (B=2, T=2048, C=768, H=12) on 8 TRN2 NeuronCores.

Sharding: core c -> batch b = c//4, head-group g = c%4 (heads 3g..3g+2).
Each core computes QKV for its 3 heads, causal attention, and a partial
c_proj (its heads' rows of W_proj). Host sums the 4 partials per batch.

Device layout is fully transposed (feature dim on partitions):
  xT [128,6,2048], qkv^T tiles [128, 2048], scores S^T [k, q], y^T, out^T.
Softmax over k (= partition dim of S^T) uses an appended ones-column on V:
the PV matmul then yields [y_unnorm^T; denom] in one accumulation group.
No max-subtraction: scores are ~N(0,1) (|s| < ~7), exp is fp32-safe.

qkv m-tile packing (host must match); 5 tiles, no zero padding:
  m0: [V_h0 | V_h1]   m1: [Q_h2 | V_h2]   m2: [Q_h0 | Q_h1]
  m3: [K_h0 | K_h1]   m4: [K_h2] (64 wide)
Q_h and K_h of each head sit at the same base partition (matmul requires
equal lhsT/rhs base partitions): Q2 base 0 in m1, K2 base 0 in m4.

Schedule: a small prework block (V01+Q01+K01 chunk 0 + first V transpose)
starts the exp stream as early as possible; ALL remaining work (other QKV
chunks, V transposes, c_proj) is drained from a work queue one item per
attention pair, so the PE queue stays dense while ACT runs the exps (and
the PE p-state ramps to full clock). Normalize uses reciprocal_approx_fast
(single custom-DVE op, ~5x faster than InstReciprocal). DMAs are batched
(~600ns issue each on the sync sequencer, so fewer+bigger is critical).
Output partials are stored bf16 and DMA'd once per 512-wide q chunk.
"""

import numpy as np
import ml_dtypes

import concourse.bass as bass
import concourse.mybir as mybir
import concourse.tile as tile
from concourse import bacc
from concourse.bass_utils import run_bass_kernel_spmd
from concourse.masks import make_identity, make_upper_triangular

F32 = mybir.dt.float32
F32R = mybir.dt.float32r
BF16 = mybir.dt.bfloat16
AF = mybir.ActivationFunctionType

T = 2048           # sequence length
C = 768            # embed dim
HPC = 3            # heads per core
D = 64             # head dim
W576 = HPC * 3 * D  # 576 qkv cols per core
QC = 512           # q-chunk (psum bank width in fp32)
KT = 128           # k-tile
NKT = T // KT      # 16
NQC = T // QC      # 4
NCH = C // 128     # 6 contraction chunks for qkv
SCALE = 1.0 / 8.0  # 1/sqrt(64)
M_W = [128, 128, 128, 128, 64]
M_OFF = [0, 128, 256, 384, 512]
USE_FAST_RECIP = True

_CACHE = {}
LAST_RESULTS = None
_TCNT = [0]


def mk_persist(pool, shape, dtype, name=None):
    if name is None:
        _TCNT[0] += 1
        name = f"pt{_TCNT[0]}"
    return pool.tile(shape, dtype, name=name, tag=name)


def build():
    nc = bacc.Bacc("TRN2", target_bir_lowering=False)

    xTb = nc.dram_tensor("xTb", [128, NCH, T], BF16, kind="ExternalInput")
    wqkvb = nc.dram_tensor("wqkvb", [128, NCH, W576], BF16, kind="ExternalInput")
    battn = nc.dram_tensor("battn", [128, 5], F32, kind="ExternalInput")
    wproj = nc.dram_tensor("wproj", [128, 2, C], BF16, kind="ExternalInput")
    yTb = nc.dram_tensor("yTb", [128, NCH, T], BF16, kind="ExternalOutput")

    with tile.TileContext(nc) as tc, \
            tc.tile_pool(name="persist", bufs=1) as pp:
        # ---- persistent SBUF tensors ----
        ident_s = mk_persist(pp, [128, 128], F32)
        trimask_s = mk_persist(pp, [128, 128], F32)  # [k, q] = 1.0 iff k <= q
        make_identity(nc, ident_s[:, :])
        make_upper_triangular(nc, trimask_s[:, :], val=1.0, diag=True)
        ident = mk_persist(pp, [128, 128], F32R)
        trimask = mk_persist(pp, [128, 128], BF16)
        nc.vector.tensor_copy(ident[:, :], ident_s[:, :])
        nc.vector.tensor_copy(trimask[:, :], trimask_s[:, :])

        # input tiles split per DMA so readers only wait on their own region
        xsb0 = mk_persist(pp, [128, NCH, QC], BF16)       # x^T chunk t=0
        xsbR = mk_persist(pp, [128, NCH, T - QC], BF16)   # x^T chunks t=1..3
        wqA = mk_persist(pp, [128, NCH, 256], BF16)       # m0, m1 cols
        wqB = mk_persist(pp, [128, NCH, W576 - 256], BF16)  # m2, m3, m4 cols
        battn_sb = mk_persist(pp, [128, 5], F32)
        wproj_sb = mk_persist(pp, [128, 2, C], BF16)  # [0]: rows 0:128, [1]: 128:192
        v01 = mk_persist(pp, [128, T], F32R)   # m0: [V_h0 | V_h1]
        q2v2 = mk_persist(pp, [128, T], F32R)  # m1: [Q_h2 | V_h2]
        qA = mk_persist(pp, [128, T], F32R)    # m2: [Q_h0 | Q_h1]
        kA = mk_persist(pp, [128, T], F32R)    # m3: [K_h0 | K_h1]
        k2 = mk_persist(pp, [64, T], F32R)     # m4: [K_h2]
        vaug = mk_persist(pp, [128, NKT * HPC, 65], BF16)  # V tiles + ones col
        yA = mk_persist(pp, [128, T], BF16)    # y^T heads 0,1
        yB = mk_persist(pp, [64, T], BF16)     # y^T head 2

        # DMA waits are a counting semaphore over all dma_starts emitted
        # before the consumer, so only the prework's inputs go first; the
        # rest are emitted after the prework (see below).
        nc.sync.dma_start(wqA[:, :, :], wqkvb[:, :, 0:256])
        nc.sync.dma_start(xsb0[:, :, :], xTb[:, :, 0:QC])
        nc.sync.dma_start(battn_sb[:, :], battn[:, :])

        def x_chunk(cc, t):
            if t == 0:
                return xsb0[:, cc, :]
            return xsbR[:, cc, (t - 1) * QC:t * QC]

        def w_cols(m):
            off = M_OFF[m]
            if off < 256:
                return wqA, off
            return wqB, off - 256

        qkv_dest = [v01, q2v2, qA, kA, k2]
        # per head: (Q tile, K tile, base row); V^T source (tile, base row)
        qk_of = [(qA, kA, 0), (qA, kA, 64), (q2v2, k2, 0)]
        vt_of = [(v01, 0), (v01, 64), (q2v2, 64)]

        with (
            tc.tile_pool(name="psA", bufs=2, space="PSUM") as psA,
            tc.tile_pool(name="psB", bufs=2, space="PSUM") as psB,
            tc.tile_pool(name="psY", bufs=2, space="PSUM") as psY,
            tc.tile_pool(name="sb", bufs=8) as sbp,
            tc.tile_pool(name="osb", bufs=2) as osbp,
        ):
            # single strided memset for every vaug ones-column
            nc.vector.memset(vaug[:, :, 64:65], 1.0)

            def emit_qkv(m, t):
                M = M_W[m]
                dest = qkv_dest[m]
                wt, woff = w_cols(m)
                ps = psB.tile([128, QC], F32, tag="psb", name="psb")
                for cc in range(NCH):
                    nc.tensor.matmul(
                        ps[:M, :],
                        lhsT=wt[:, cc, woff:woff + M],
                        rhs=x_chunk(cc, t),
                        start=(cc == 0), stop=(cc == NCH - 1),
                    )
                nc.vector.tensor_scalar_add(
                    dest[:M, t * QC:(t + 1) * QC], ps[:M, :],
                    battn_sb[:M, m:m + 1],
                )

            def emit_vtrans(h, kt4):
                vsrc, vb = vt_of[h]
                pt = psB.tile([128, QC], F32, tag="psb", name="psb")
                for j in range(4):
                    kt = kt4 * 4 + j
                    nc.tensor.transpose(
                        pt[:, j * 64:(j + 1) * 64].bitcast(F32R),
                        vsrc[vb:vb + 64, kt * KT:(kt + 1) * KT],
                        ident[vb:vb + 64, vb:vb + 64],
                    )
                vi = h * NKT + kt4 * 4
                nc.vector.tensor_copy(
                    vaug[:, vi:vi + 4, 0:64],
                    pt[:, 0:256].rearrange("p (a b) -> p a b", b=64),
                )

            def emit_attn(h, t, drain):
                qt, kt_t, qb = qk_of[h]
                ydest, yrow = (yA, 0) if h == 0 else (yA, 64) if h == 1 else (yB, 0)
                qlo_g = t * QC
                py = psY.tile([128, QC], F32, tag="py", name="py")
                n_k = 4 * (t + 1)
                n_pair = n_k // 2

                def qlo_of(kt):
                    dm = kt - 4 * t
                    return 128 * dm if dm >= 0 else 0

                def emit_S(p):
                    ps = psA.tile([128, 2 * QC], F32, tag="ps", name="ps")
                    pT = sbp.tile([128, 2 * QC], BF16, tag="pT", name="pT")
                    for half in range(2):
                        kt = 2 * p + half
                        qlo = qlo_of(kt)
                        nc.tensor.matmul(
                            ps[:, half * QC + qlo:(half + 1) * QC],
                            lhsT=kt_t[qb:qb + 64, kt * KT:(kt + 1) * KT],
                            rhs=qt[qb:qb + 64, qlo_g + qlo:qlo_g + QC],
                            start=True, stop=True,
                        )
                    lo0, lo1 = qlo_of(2 * p), qlo_of(2 * p + 1)
                    if lo1 == 0:
                        nc.scalar.activation(
                            pT[:, 0:2 * QC], ps[:, 0:2 * QC], AF.Exp,
                            scale=SCALE,
                        )
                    else:
                        # diagonal pair: only exp the psum regions S wrote
                        nc.scalar.activation(
                            pT[:, lo0:QC], ps[:, lo0:QC], AF.Exp,
                            scale=SCALE,
                        )
                        nc.scalar.activation(
                            pT[:, QC + lo1:2 * QC], ps[:, QC + lo1:2 * QC],
                            AF.Exp, scale=SCALE,
                        )
                    for half in range(2):
                        kt = 2 * p + half
                        if kt - 4 * t >= 0:
                            o = half * QC + qlo_of(kt)
                            nc.vector.tensor_mul(
                                pT[:, o:o + 128], pT[:, o:o + 128],
                                trimask[:, :],
                            )
                    return pT

                def emit_PV(p, pT):
                    for half in range(2):
                        kt = 2 * p + half
                        qlo = qlo_of(kt)
                        nc.tensor.matmul(
                            py[0:65, qlo:QC],
                            lhsT=vaug[:, h * NKT + kt, :],
                            rhs=pT[:, half * QC + qlo:(half + 1) * QC],
                            start=(kt == 0), stop=(kt == n_k - 1),
                        )

                pTs = {0: emit_S(0)}
                for p in range(n_pair):
                    if p + 1 < n_pair:
                        pTs[p + 1] = emit_S(p + 1)
                    emit_PV(p, pTs.pop(p))
                    drain(1)

                # fast approximate reciprocal of the denom row (psum in),
                # broadcast to 64 lanes on gpsimd, multiply on DVE.
                rec = sbp.tile([1, QC], F32, tag="rec", name="rec")
                den = sbp.tile([1, QC], F32, tag="den", name="den")
                nc.vector.tensor_copy(den[0:1, :], py[64:65, :])
                if USE_FAST_RECIP:
                    nc.vector.reciprocal_approx_fast(rec[0:1, :], den[0:1, :])
                else:
                    nc.vector.reciprocal(rec[0:1, :], den[0:1, :])
                bcast = sbp.tile([64, QC], F32, tag="bcast", name="bcast")
                nc.gpsimd.partition_broadcast(bcast[:, :], rec[0:1, :])
                nc.vector.tensor_mul(
                    ydest[yrow:yrow + 64, qlo_g:qlo_g + QC],
                    py[0:64, :], bcast[:, :],
                )

            osb_of = {}

            def emit_proj(ct, t):
                if t not in osb_of:
                    osb_of[t] = osbp.tile([128, NCH, QC], BF16, tag="osb",
                                          name="osb")
                osb = osb_of[t]
                ps = psB.tile([128, QC], F32, tag="psb", name="psb")
                nc.tensor.matmul(
                    ps[:, :],
                    lhsT=wproj_sb[:, 0, ct * 128:(ct + 1) * 128],
                    rhs=yA[:, t * QC:(t + 1) * QC],
                    start=True, stop=False,
                )
                nc.tensor.matmul(
                    ps[:, :],
                    lhsT=wproj_sb[0:64, 1, ct * 128:(ct + 1) * 128],
                    rhs=yB[0:64, t * QC:(t + 1) * QC],
                    start=False, stop=True,
                )
                nc.scalar.activation(osb[:, ct, :], ps[:, :], AF.Copy)
                if ct % 2 == 1:
                    # issue the output DMA per ct-pair to overlap transfer
                    nc.sync.dma_start(
                        yTb[:, ct - 1:ct + 1, t * QC:(t + 1) * QC],
                        osb[:, ct - 1:ct + 1, :])

            # ---- work queue: everything not in prework, dependency order.
            WQ = []
            done = set()
            idx = [0]

            def push(tag, fn, *a):
                WQ.append((tag, fn, a))

            def drain(n):
                for _ in range(n):
                    if idx[0] >= len(WQ):
                        return
                    tag, fn, a = WQ[idx[0]]
                    idx[0] += 1
                    fn(*a)
                    done.add(tag)

            def ensure(*tags):
                want = set(tags) - done
                while want:
                    assert idx[0] < len(WQ), f"unsatisfiable prereqs {want}"
                    drain(1)
                    want -= done

            # t=0 extras
            push("qkv1_0", emit_qkv, 1, 0)
            push("vt1_0", emit_vtrans, 1, 0)
            push("vt2_0", emit_vtrans, 2, 0)
            push("qkv4_0", emit_qkv, 4, 0)
            for t in range(1, NQC):
                push(f"qkv0_{t}", emit_qkv, 0, t)
                push(f"qkv2_{t}", emit_qkv, 2, t)
                push(f"qkv3_{t}", emit_qkv, 3, t)
                push(f"vt0_{t}", emit_vtrans, 0, t)
                push(f"qkv1_{t}", emit_qkv, 1, t)
                push(f"vt1_{t}", emit_vtrans, 1, t)
                push(f"vt2_{t}", emit_vtrans, 2, t)
                push(f"qkv4_{t}", emit_qkv, 4, t)

            # ---- prework: get the first attention block running ASAP
            emit_qkv(0, 0)       # V01 chunk 0
            emit_vtrans(0, 0)
            nc.sync.dma_start(wqB[:, :, :], wqkvb[:, :, 256:W576])
            emit_qkv(2, 0)       # Q01 chunk 0
            emit_qkv(3, 0)       # K01 chunk 0
            # bulk inputs last: nothing before this point waits on them
            for t in range(1, NQC):
                nc.sync.dma_start(
                    xsbR[:, :, (t - 1) * QC:t * QC],
                    xTb[:, :, t * QC:(t + 1) * QC])
            nc.sync.dma_start(wproj_sb[:, :, :], wproj[:, :, :])

            prereq = [
                lambda t: [f"qkv2_{t}", f"qkv3_{t}", f"vt0_{t}"],
                lambda t: [f"qkv2_{t}", f"qkv3_{t}", f"vt1_{t}"],
                lambda t: [f"qkv1_{t}", f"qkv4_{t}", f"vt2_{t}"],
            ]
            done.update(["qkv0_0", "vt0_0", "qkv2_0", "qkv3_0"])

            for t in range(NQC):
                for h in range(HPC):
                    ensure(*[p for p in prereq[h](t) if p not in done])
                    emit_attn(h, t, drain)
                for ct in range(NCH):
                    push(f"proj_{ct}_{t}", emit_proj, ct, t)
            drain(len(WQ))

    nc.finalize()
    return nc


def kernel(x, W_attn, b_attn, W_proj, b_proj):
    global LAST_RESULTS
    B = x.shape[0]
    x = np.asarray(x, np.float32)
    W_attn = np.asarray(W_attn, np.float32)
    b_attn = np.asarray(b_attn, np.float32)
    W_proj = np.asarray(W_proj, np.float32)
    b_proj = np.asarray(b_proj, np.float32)

    if "nc" not in _CACHE:
        _CACHE["nc"] = build()
    nc = _CACHE["nc"]

    in_maps = []
    for c in range(8):
        b, g = divmod(c, 4)
        heads = [3 * g + i for i in range(HPC)]
        h0, h1, h2 = heads
        Q = lambda h: W_attn[:, 64 * h:64 * h + 64]
        K = lambda h: W_attn[:, C + 64 * h:C + 64 * h + 64]
        V = lambda h: W_attn[:, 2 * C + 64 * h:2 * C + 64 * h + 64]
        bQ = lambda h: b_attn[64 * h:64 * h + 64]
        bK = lambda h: b_attn[C + 64 * h:C + 64 * h + 64]
        bV = lambda h: b_attn[2 * C + 64 * h:2 * C + 64 * h + 64]
        # m-tiles: [V0|V1], [Q2|V2], [Q0|Q1], [K0|K1], [K2]
        wq = np.concatenate(
            [V(h0), V(h1), Q(h2), V(h2), Q(h0), Q(h1), K(h0), K(h1), K(h2)],
            1)                                            # [768, 576]
        wqkvb = np.ascontiguousarray(
            wq.reshape(NCH, 128, W576).transpose(1, 0, 2)
        ).astype(ml_dtypes.bfloat16)                      # [128, 6, 576]
        bcols = [bV(h0), bV(h1), bQ(h2), bV(h2), bQ(h0), bQ(h1),
                 bK(h0), bK(h1), bK(h2), np.zeros(64, np.float32)]
        bvec = np.concatenate(bcols)                      # [640] = 5 x 128
        battn = np.ascontiguousarray(bvec.reshape(5, 128).T)  # [128, 5]
        wp = np.zeros((256, C), np.float32)
        wp[:192] = np.concatenate(
            [W_proj[64 * h:64 * h + 64, :] for h in heads], 0)
        wproj = np.ascontiguousarray(
            wp.reshape(2, 128, C).transpose(1, 0, 2)
        ).astype(ml_dtypes.bfloat16)                      # [128, 2, 768]
        xt = np.ascontiguousarray(
            x[b].T.reshape(NCH, 128, T).transpose(1, 0, 2))
        in_maps.append({
            "xTb": xt.astype(ml_dtypes.bfloat16),
            "wqkvb": wqkvb,
            "battn": battn,
            "wproj": wproj,
        })

    res = run_bass_kernel_spmd(nc, in_maps, core_ids=list(range(8)))
    LAST_RESULTS = res

    out = np.zeros((B, T, C), np.float32)
    for c in range(8):
        b = c // 4
        yT = res.results[c]["yTb"].astype(np.float32)     # [128, 6, 2048]
        out[b] += yT.transpose(1, 0, 2).reshape(C, T).T
    out += b_proj
    return out


# revision 21
# speedup vs baseline: 1.0171x; 1.0171x over previous
"""Causal self-attention (B=2, T=2048, C=768, H=12) on 8 TRN2 NeuronCores.

Sharding: core c -> batch b = c//4, head-group g = c%4 (heads 3g..3g+2).
Each core computes QKV for its 3 heads, causal attention, and a partial
c_proj (its heads' rows of W_proj). Host sums the 4 partials per batch.

Device layout is fully transposed (feature dim on partitions):
  xT [128,6,2048], qkv^T tiles [128, 2048], scores S^T [k, q], y^T, out^T.
Softmax over k (= partition dim of S^T) uses an appended ones-column on V:
the PV matmul then yields [y_unnorm^T; denom] in one accumulation group.
No max-subtraction: scores are ~N(0,1) (|s| < ~7), exp is fp32-safe.

qkv m-tile packing (host must match); 5 tiles, no zero padding:
  m0: [V_h0 | V_h1]   m1: [Q_h2 | V_h2]   m2: [Q_h0 | Q_h1]
  m3: [K_h0 | K_h1]   m4: [K_h2] (64 wide)
Q_h and K_h of each head sit at the same base partition (matmul requires
equal lhsT/rhs base partitions): Q2 base 0 in m1, K2 base 0 in m4.

Schedule: a small prework block (V01+Q01+K01 chunk 0 + first V transpose)
starts the exp stream as early as possible; ALL remaining work (other QKV
chunks, V transposes, c_proj) is drained from a work queue one item per
attention pair, so the PE queue stays dense while ACT runs the exps (and
the PE p-state ramps to full clock). Normalize uses reciprocal_approx_fast
(single custom-DVE op, ~5x faster than InstReciprocal). DMAs are batched
(~600ns issue each on the sync sequencer, so fewer+bigger is critical).
Output partials are stored bf16 and DMA'd once per 512-wide q chunk.
"""

import numpy as np
import ml_dtypes

import concourse.bass as bass
import concourse.mybir as mybir
import concourse.tile as tile
from concourse import bacc
from concourse.bass_utils import run_bass_kernel_spmd
from concourse.masks import make_identity, make_upper_triangular

F32 = mybir.dt.float32
F32R = mybir.dt.float32r
BF16 = mybir.dt.bfloat16
AF = mybir.ActivationFunctionType

T = 2048           # sequence length
C = 768            # embed dim
HPC = 3            # heads per core
D = 64             # head dim
W576 = HPC * 3 * D  # 576 qkv cols per core
QC = 512           # q-chunk (psum bank width in fp32)
KT = 128           # k-tile
NKT = T // KT      # 16
NQC = T // QC      # 4
NCH = C // 128     # 6 contraction chunks for qkv
SCALE = 1.0 / 8.0  # 1/sqrt(64)
M_W = [128, 128, 128, 128, 64]
M_OFF = [0, 128, 256, 384, 512]
USE_FAST_RECIP = True

_CACHE = {}
LAST_RESULTS = None
_TCNT = [0]


def mk_persist(pool, shape, dtype, name=None):
    if name is None:
        _TCNT[0] += 1
        name = f"pt{_TCNT[0]}"
    return pool.tile(shape, dtype, name=name, tag=name)


def build():
    nc = bacc.Bacc("TRN2", target_bir_lowering=False)

    # x is t-chunk-major and wqkv m-tile-major so each prework DMA reads
    # one contiguous run per partition (128 fat descriptors, fast init)
    xTb = nc.dram_tensor("xTb", [128, NQC, NCH, QC], BF16, kind="ExternalInput")
    wqkvb = nc.dram_tensor("wqkvb", [128, 5, NCH, 128], BF16, kind="ExternalInput")
    battn = nc.dram_tensor("battn", [128, 5], F32, kind="ExternalInput")
    wproj = nc.dram_tensor("wproj", [128, 2, C], BF16, kind="ExternalInput")
    yTb = nc.dram_tensor("yTb", [128, NCH, T], BF16, kind="ExternalOutput")

    with tile.TileContext(nc) as tc, \
            tc.tile_pool(name="persist", bufs=1) as pp:
        # ---- persistent SBUF tensors ----
        ident_s = mk_persist(pp, [128, 128], F32)
        trimask_s = mk_persist(pp, [128, 128], F32)  # [k, q] = 1.0 iff k <= q
        make_identity(nc, ident_s[:, :])
        make_upper_triangular(nc, trimask_s[:, :], val=1.0, diag=True)
        ident = mk_persist(pp, [128, 128], F32R)
        trimask = mk_persist(pp, [128, 128], BF16)
        nc.vector.tensor_copy(ident[:, :], ident_s[:, :])
        nc.vector.tensor_copy(trimask[:, :], trimask_s[:, :])

        # input tiles split per DMA so readers only wait on their own region
        xsb_t = [mk_persist(pp, [128, NCH, QC], BF16) for _ in range(NQC)]
        wq_m = [mk_persist(pp, [128, NCH, 128], BF16) for _ in range(5)]
        battn_sb = mk_persist(pp, [128, 5], F32)
        wproj_sb = mk_persist(pp, [128, 2, C], BF16)  # [0]: rows 0:128, [1]: 128:192
        v01 = mk_persist(pp, [128, T], F32R)   # m0: [V_h0 | V_h1]
        q2v2 = mk_persist(pp, [128, T], F32R)  # m1: [Q_h2 | V_h2]
        qA = mk_persist(pp, [128, T], F32R)    # m2: [Q_h0 | Q_h1]
        kA = mk_persist(pp, [128, T], F32R)    # m3: [K_h0 | K_h1]
        k2 = mk_persist(pp, [64, T], F32R)     # m4: [K_h2]
        vaug = mk_persist(pp, [128, NKT * HPC, 65], BF16)  # V tiles + ones col
        yA = mk_persist(pp, [128, T], BF16)    # y^T heads 0,1
        yB = mk_persist(pp, [64, T], BF16)     # y^T head 2

        # The first matmul's DMA wait covers every dma_start emitted before
        # it, and transfers drain FIFO — so only the prework's three inputs
        # go first; everything else is emitted after the prework.
        nc.sync.dma_start(battn_sb[:, :], battn[:, :])
        nc.sync.dma_start(wq_m[0][:, :, :], wqkvb[:, 0, :, :])
        nc.sync.dma_start(xsb_t[0][:, :, :], xTb[:, 0, :, :])

        qkv_dest = [v01, q2v2, qA, kA, k2]
        # per head: (Q tile, K tile, base row); V^T source (tile, base row)
        qk_of = [(qA, kA, 0), (qA, kA, 64), (q2v2, k2, 0)]
        vt_of = [(v01, 0), (v01, 64), (q2v2, 64)]

        with (
            tc.tile_pool(name="psA", bufs=2, space="PSUM") as psA,
            tc.tile_pool(name="psB", bufs=2, space="PSUM") as psB,
            tc.tile_pool(name="psY", bufs=2, space="PSUM") as psY,
            tc.tile_pool(name="sb", bufs=8) as sbp,
            tc.tile_pool(name="osb", bufs=2) as osbp,
        ):
            # single strided memset for every vaug ones-column
            nc.vector.memset(vaug[:, :, 64:65], 1.0)

            def emit_qkv(m, t):
                M = M_W[m]
                dest = qkv_dest[m]
                ps = psB.tile([128, QC], F32, tag="psb", name="psb")
                for cc in range(NCH):
                    nc.tensor.matmul(
                        ps[:M, :],
                        lhsT=wq_m[m][:, cc, 0:M],
                        rhs=xsb_t[t][:, cc, :],
                        start=(cc == 0), stop=(cc == NCH - 1),
                    )
                nc.vector.tensor_scalar_add(
                    dest[:M, t * QC:(t + 1) * QC], ps[:M, :],
                    battn_sb[:M, m:m + 1],
                )

            def emit_vtrans(h, kt4):
                vsrc, vb = vt_of[h]
                pt = psB.tile([128, QC], F32, tag="psb", name="psb")
                for j in range(4):
                    kt = kt4 * 4 + j
                    nc.tensor.transpose(
                        pt[:, j * 64:(j + 1) * 64].bitcast(F32R),
                        vsrc[vb:vb + 64, kt * KT:(kt + 1) * KT],
                        ident[vb:vb + 64, vb:vb + 64],
                    )
                vi = h * NKT + kt4 * 4
                nc.vector.tensor_copy(
                    vaug[:, vi:vi + 4, 0:64],
                    pt[:, 0:256].rearrange("p (a b) -> p a b", b=64),
                )

            def emit_attn(h, t, drain):
                qt, kt_t, qb = qk_of[h]
                ydest, yrow = (yA, 0) if h == 0 else (yA, 64) if h == 1 else (yB, 0)
                qlo_g = t * QC
                py = psY.tile([128, QC], F32, tag="py", name="py")
                n_k = 4 * (t + 1)
                n_pair = n_k // 2

                def qlo_of(kt):
                    dm = kt - 4 * t
                    return 128 * dm if dm >= 0 else 0

                def emit_S(p):
                    ps = psA.tile([128, 2 * QC], F32, tag="ps", name="ps")
                    pT = sbp.tile([128, 2 * QC], BF16, tag="pT", name="pT")
                    for half in range(2):
                        kt = 2 * p + half
                        qlo = qlo_of(kt)
                        nc.tensor.matmul(
                            ps[:, half * QC + qlo:(half + 1) * QC],
                            lhsT=kt_t[qb:qb + 64, kt * KT:(kt + 1) * KT],
                            rhs=qt[qb:qb + 64, qlo_g + qlo:qlo_g + QC],
                            start=True, stop=True,
                        )
                    lo0, lo1 = qlo_of(2 * p), qlo_of(2 * p + 1)
                    if lo1 == 0:
                        nc.scalar.activation(
                            pT[:, 0:2 * QC], ps[:, 0:2 * QC], AF.Exp,
                            scale=SCALE,
                        )
                    else:
                        # diagonal pair: only exp the psum regions S wrote
                        nc.scalar.activation(
                            pT[:, lo0:QC], ps[:, lo0:QC], AF.Exp,
                            scale=SCALE,
                        )
                        nc.scalar.activation(
                            pT[:, QC + lo1:2 * QC], ps[:, QC + lo1:2 * QC],
                            AF.Exp, scale=SCALE,
                        )
                    for half in range(2):
                        kt = 2 * p + half
                        if kt - 4 * t >= 0:
                            o = half * QC + qlo_of(kt)
                            nc.vector.tensor_mul(
                                pT[:, o:o + 128], pT[:, o:o + 128],
                                trimask[:, :],
                            )
                    return pT

                def emit_PV(p, pT):
                    for half in range(2):
                        kt = 2 * p + half
                        qlo = qlo_of(kt)
                        nc.tensor.matmul(
                            py[0:65, qlo:QC],
                            lhsT=vaug[:, h * NKT + kt, :],
                            rhs=pT[:, half * QC + qlo:(half + 1) * QC],
                            start=(kt == 0), stop=(kt == n_k - 1),
                        )

                pTs = {0: emit_S(0)}
                for p in range(n_pair):
                    if p + 1 < n_pair:
                        pTs[p + 1] = emit_S(p + 1)
                    emit_PV(p, pTs.pop(p))
                    drain(1)

                # fast approximate reciprocal of the denom row (psum in),
                # broadcast to 64 lanes on gpsimd, multiply on DVE.
                rec = sbp.tile([1, QC], F32, tag="rec", name="rec")
                den = sbp.tile([1, QC], F32, tag="den", name="den")
                nc.vector.tensor_copy(den[0:1, :], py[64:65, :])
                if USE_FAST_RECIP:
                    nc.vector.reciprocal_approx_fast(rec[0:1, :], den[0:1, :])
                else:
                    nc.vector.reciprocal(rec[0:1, :], den[0:1, :])
                bcast = sbp.tile([64, QC], F32, tag="bcast", name="bcast")
                nc.gpsimd.partition_broadcast(bcast[:, :], rec[0:1, :])
                nc.vector.tensor_mul(
                    ydest[yrow:yrow + 64, qlo_g:qlo_g + QC],
                    py[0:64, :], bcast[:, :],
                )

            osb_of = {}

            def emit_proj(ct, t):
                if t not in osb_of:
                    osb_of[t] = osbp.tile([128, NCH, QC], BF16, tag="osb",
                                          name="osb")
                osb = osb_of[t]
                ps = psB.tile([128, QC], F32, tag="psb", name="psb")
                nc.tensor.matmul(
                    ps[:, :],
                    lhsT=wproj_sb[:, 0, ct * 128:(ct + 1) * 128],
                    rhs=yA[:, t * QC:(t + 1) * QC],
                    start=True, stop=False,
                )
                nc.tensor.matmul(
                    ps[:, :],
                    lhsT=wproj_sb[0:64, 1, ct * 128:(ct + 1) * 128],
                    rhs=yB[0:64, t * QC:(t + 1) * QC],
                    start=False, stop=True,
                )
                # stage on DVE: the ACT queue is deep with exps during the
                # attention phase and would hold the psB tile for too long
                nc.vector.tensor_copy(osb[:, ct, :], ps[:, :])
                if ct % 2 == 1:
                    # issue the output DMA per ct-pair to overlap transfer
                    nc.sync.dma_start(
                        yTb[:, ct - 1:ct + 1, t * QC:(t + 1) * QC],
                        osb[:, ct - 1:ct + 1, :])

            # ---- work queue: everything not in prework, dependency order.
            WQ = []
            done = set()
            idx = [0]

            def push(tag, fn, *a):
                WQ.append((tag, fn, a))

            def drain(n):
                for _ in range(n):
                    if idx[0] >= len(WQ):
                        return
                    tag, fn, a = WQ[idx[0]]
                    idx[0] += 1
                    fn(*a)
                    done.add(tag)

            def ensure(*tags):
                want = set(tags) - done
                while want:
                    assert idx[0] < len(WQ), f"unsatisfiable prereqs {want}"
                    drain(1)
                    want -= done

            # t=0 extras
            push("qkv1_0", emit_qkv, 1, 0)
            push("vt1_0", emit_vtrans, 1, 0)
            push("vt2_0", emit_vtrans, 2, 0)
            push("qkv4_0", emit_qkv, 4, 0)
            for t in range(1, NQC):
                push(f"qkv0_{t}", emit_qkv, 0, t)
                push(f"qkv2_{t}", emit_qkv, 2, t)
                push(f"qkv3_{t}", emit_qkv, 3, t)
                push(f"vt0_{t}", emit_vtrans, 0, t)
                push(f"qkv1_{t}", emit_qkv, 1, t)
                push(f"vt1_{t}", emit_vtrans, 1, t)
                push(f"vt2_{t}", emit_vtrans, 2, t)
                push(f"qkv4_{t}", emit_qkv, 4, t)

            # ---- prework: get the first attention block running ASAP
            emit_qkv(0, 0)       # V01 chunk 0
            emit_vtrans(0, 0)
            nc.sync.dma_start(wq_m[2][:, :, :], wqkvb[:, 2, :, :])
            nc.sync.dma_start(wq_m[3][:, :, :], wqkvb[:, 3, :, :])
            emit_qkv(2, 0)       # Q01 chunk 0
            emit_qkv(3, 0)       # K01 chunk 0
            # bulk inputs last: nothing before this point waits on them
            nc.sync.dma_start(wq_m[1][:, :, :], wqkvb[:, 1, :, :])
            nc.sync.dma_start(wq_m[4][:, :, :], wqkvb[:, 4, :, :])
            for t in range(1, NQC):
                nc.sync.dma_start(xsb_t[t][:, :, :], xTb[:, t, :, :])
            nc.sync.dma_start(wproj_sb[:, :, :], wproj[:, :, :])

            prereq = [
                lambda t: [f"qkv2_{t}", f"qkv3_{t}", f"vt0_{t}"],
                lambda t: [f"qkv2_{t}", f"qkv3_{t}", f"vt1_{t}"],
                lambda t: [f"qkv1_{t}", f"qkv4_{t}", f"vt2_{t}"],
            ]
            done.update(["qkv0_0", "vt0_0", "qkv2_0", "qkv3_0"])

            for t in range(NQC):
                for h in range(HPC):
                    ensure(*[p for p in prereq[h](t) if p not in done])
                    emit_attn(h, t, drain)
                for ct in range(NCH):
                    push(f"proj_{ct}_{t}", emit_proj, ct, t)
            drain(len(WQ))

    nc.finalize()
    return nc


def kernel(x, W_attn, b_attn, W_proj, b_proj):
    global LAST_RESULTS
    B = x.shape[0]
    x = np.asarray(x, np.float32)
    W_attn = np.asarray(W_attn, np.float32)
    b_attn = np.asarray(b_attn, np.float32)
    W_proj = np.asarray(W_proj, np.float32)
    b_proj = np.asarray(b_proj, np.float32)

    if "nc" not in _CACHE:
        _CACHE["nc"] = build()
    nc = _CACHE["nc"]

    in_maps = []
    for c in range(8):
        b, g = divmod(c, 4)
        heads = [3 * g + i for i in range(HPC)]
        h0, h1, h2 = heads
        Q = lambda h: W_attn[:, 64 * h:64 * h + 64]
        K = lambda h: W_attn[:, C + 64 * h:C + 64 * h + 64]
        V = lambda h: W_attn[:, 2 * C + 64 * h:2 * C + 64 * h + 64]
        bQ = lambda h: b_attn[64 * h:64 * h + 64]
        bK = lambda h: b_attn[C + 64 * h:C + 64 * h + 64]
        bV = lambda h: b_attn[2 * C + 64 * h:2 * C + 64 * h + 64]
        # m-tiles: [V0|V1], [Q2|V2], [Q0|Q1], [K0|K1], [K2] — m-major 4D
        mt = [np.concatenate([V(h0), V(h1)], 1),
              np.concatenate([Q(h2), V(h2)], 1),
              np.concatenate([Q(h0), Q(h1)], 1),
              np.concatenate([K(h0), K(h1)], 1),
              np.concatenate([K(h2), np.zeros((C, 64), np.float32)], 1)]
        wqkvb = np.zeros((128, 5, NCH, 128), np.float32)
        for m, w in enumerate(mt):
            wqkvb[:, m] = w.reshape(NCH, 128, 128).transpose(1, 0, 2)
        wqkvb = np.ascontiguousarray(wqkvb).astype(ml_dtypes.bfloat16)
        bcols = [bV(h0), bV(h1), bQ(h2), bV(h2), bQ(h0), bQ(h1),
                 bK(h0), bK(h1), bK(h2), np.zeros(64, np.float32)]
        bvec = np.concatenate(bcols)                      # [640] = 5 x 128
        battn = np.ascontiguousarray(bvec.reshape(5, 128).T)  # [128, 5]
        wp = np.zeros((256, C), np.float32)
        wp[:192] = np.concatenate(
            [W_proj[64 * h:64 * h + 64, :] for h in heads], 0)
        wproj = np.ascontiguousarray(
            wp.reshape(2, 128, C).transpose(1, 0, 2)
        ).astype(ml_dtypes.bfloat16)                      # [128, 2, 768]
        # [128, NQC, NCH, QC]: xt[p, t, cc, q] = x^T[cc*128+p, t*QC+q]
        xt = np.ascontiguousarray(
            x[b].T.reshape(NCH, 128, NQC, QC).transpose(1, 2, 0, 3))
        in_maps.append({
            "xTb": xt.astype(ml_dtypes.bfloat16),
            "wqkvb": wqkvb,
            "battn": battn,
            "wproj": wproj,
        })

    res = run_bass_kernel_spmd(nc, in_maps, core_ids=list(range(8)))
    LAST_RESULTS = res

    out = np.zeros((B, T, C), np.float32)
    for c in range(8):
        b = c // 4
        yT = res.results[c]["yTb"].astype(np.float32)     # [128, 6, 2048]
        out[b] += yT.transpose(1, 0, 2).reshape(C, T).T
    out += b_proj
    return out


# revision 26
# speedup vs baseline: 1.0650x; 1.0470x over previous
"""Causal self-attention (B=2, T=2048, C=768, H=12) on 8 TRN2 NeuronCores.

Sharding: core c -> batch b = c//4, head-group g = c%4 (heads 3g..3g+2).
Each core computes QKV for its 3 heads, causal attention, and a partial
c_proj (its heads' rows of W_proj). Host sums the 4 partials per batch.

Device layout is fully transposed (feature dim on partitions):
  xT [128,6,2048], qkv^T tiles [128, 2048], scores S^T [k, q], y^T, out^T.
Softmax over k (= partition dim of S^T) uses an appended ones-column on V:
the PV matmul then yields [y_unnorm^T; denom] in one accumulation group.
No max-subtraction: scores are ~N(0,1) (|s| < ~7), exp is fp32-safe.

qkv m-tile packing (host must match); 5 tiles, no zero padding:
  m0: [V_h0 | V_h1]   m1: [Q_h2 | V_h2]   m2: [Q_h0 | Q_h1]
  m3: [K_h0 | K_h1]   m4: [K_h2] (64 wide)
Q_h and K_h of each head sit at the same base partition (matmul requires
equal lhsT/rhs base partitions): Q2 base 0 in m1, K2 base 0 in m4.

Schedule: a small prework block (V01+Q01+K01 chunk 0 + first V transpose)
starts the exp stream as early as possible; ALL remaining work (other QKV
chunks, V transposes, c_proj) is drained from a work queue one item per
attention pair, so the PE queue stays dense while ACT runs the exps (and
the PE p-state ramps to full clock). Normalize uses reciprocal_approx_fast
(single custom-DVE op, ~5x faster than InstReciprocal). DMAs are batched
(~600ns issue each on the sync sequencer, so fewer+bigger is critical).
Output partials are stored bf16 and DMA'd once per 512-wide q chunk.
"""

import numpy as np
import ml_dtypes

import concourse.bass as bass
import concourse.mybir as mybir
import concourse.tile as tile
from concourse import bacc
from concourse.bass_utils import run_bass_kernel_spmd
from concourse.masks import make_identity, make_upper_triangular

F32 = mybir.dt.float32
F32R = mybir.dt.float32r
BF16 = mybir.dt.bfloat16
AF = mybir.ActivationFunctionType

T = 2048           # sequence length
C = 768            # embed dim
HPC = 3            # heads per core
D = 64             # head dim
W576 = HPC * 3 * D  # 576 qkv cols per core
QC = 512           # q-chunk (psum bank width in fp32)
KT = 128           # k-tile
NKT = T // KT      # 16
NQC = T // QC      # 4
NCH = C // 128     # 6 contraction chunks for qkv
SCALE = 1.0 / 8.0  # 1/sqrt(64)
M_W = [128, 128, 128, 128, 64]
M_OFF = [0, 128, 256, 384, 512]
USE_FAST_RECIP = True

_CACHE = {}
LAST_RESULTS = None
_TCNT = [0]


def mk_persist(pool, shape, dtype, name=None):
    if name is None:
        _TCNT[0] += 1
        name = f"pt{_TCNT[0]}"
    return pool.tile(shape, dtype, name=name, tag=name)


def build():
    nc = bacc.Bacc("TRN2", target_bir_lowering=False)

    # x is t-chunk-major and wqkv m-tile-major so each prework DMA reads
    # one contiguous run per partition (128 fat descriptors, fast init)
    xTb = nc.dram_tensor("xTb", [128, NQC, NCH, QC], BF16, kind="ExternalInput")
    wqkvb = nc.dram_tensor("wqkvb", [128, 5, NCH, 128], BF16, kind="ExternalInput")
    battn = nc.dram_tensor("battn", [128, 5], F32, kind="ExternalInput")
    wproj = nc.dram_tensor("wproj", [128, 2, C], BF16, kind="ExternalInput")
    yTb = nc.dram_tensor("yTb", [128, NCH, T], BF16, kind="ExternalOutput")

    with tile.TileContext(nc) as tc, \
            tc.tile_pool(name="persist", bufs=1) as pp:
        # ---- persistent SBUF tensors ----
        ident_s = mk_persist(pp, [128, 128], F32)
        trimask_s = mk_persist(pp, [128, 128], F32)  # [k, q] = 1.0 iff k <= q
        make_identity(nc, ident_s[:, :])
        make_upper_triangular(nc, trimask_s[:, :], val=1.0, diag=True)
        ident = mk_persist(pp, [128, 128], F32R)
        trimask = mk_persist(pp, [128, 128], BF16)
        nc.vector.tensor_copy(ident[:, :], ident_s[:, :])
        nc.vector.tensor_copy(trimask[:, :], trimask_s[:, :])

        # input tiles split per DMA so readers only wait on their own region
        xsb_t = [mk_persist(pp, [128, NCH, QC], BF16) for _ in range(NQC)]
        wq_m = [mk_persist(pp, [128, NCH, 128], BF16) for _ in range(5)]
        battn_sb = mk_persist(pp, [128, 5], F32)
        wproj_sb = mk_persist(pp, [128, 2, C], BF16)  # [0]: rows 0:128, [1]: 128:192
        v01 = mk_persist(pp, [128, T], F32R)   # m0: [V_h0 | V_h1]
        q2v2 = mk_persist(pp, [128, T], F32R)  # m1: [Q_h2 | V_h2]
        qA = mk_persist(pp, [128, T], BF16)    # m2: [Q_h0 | Q_h1]
        kA = mk_persist(pp, [128, T], BF16)    # m3: [K_h0 | K_h1]
        k2 = mk_persist(pp, [64, T], F32R)     # m4: [K_h2]
        vaug = mk_persist(pp, [128, NKT * HPC, 65], BF16)  # V tiles + ones col
        yA = mk_persist(pp, [128, T], BF16)    # y^T heads 0,1
        yB = mk_persist(pp, [64, T], BF16)     # y^T head 2

        # The first matmul's DMA wait covers every dma_start emitted before
        # it, and transfers drain FIFO — so only the prework's three inputs
        # go first; everything else is emitted after the prework.
        nc.sync.dma_start(battn_sb[:, :], battn[:, :])
        nc.sync.dma_start(wq_m[0][:, :, :], wqkvb[:, 0, :, :])
        nc.sync.dma_start(xsb_t[0][:, :, :], xTb[:, 0, :, :])

        qkv_dest = [v01, q2v2, qA, kA, k2]
        # per head: (Q tile, K tile, base row); V^T source (tile, base row)
        qk_of = [(qA, kA, 0), (qA, kA, 64), (q2v2, k2, 0)]
        vt_of = [(v01, 0), (v01, 64), (q2v2, 64)]

        with (
            tc.tile_pool(name="psA", bufs=2, space="PSUM") as psA,
            tc.tile_pool(name="psB", bufs=2, space="PSUM") as psB,
            tc.tile_pool(name="psY", bufs=2, space="PSUM") as psY,
            tc.tile_pool(name="sb", bufs=8) as sbp,
            tc.tile_pool(name="osb", bufs=2) as osbp,
        ):
            # single strided memset for every vaug ones-column
            nc.vector.memset(vaug[:, :, 64:65], 1.0)

            def emit_qkv(m, t):
                M = M_W[m]
                dest = qkv_dest[m]
                ps = psB.tile([128, QC], F32, tag="psb", name="psb")
                for cc in range(NCH):
                    nc.tensor.matmul(
                        ps[:M, :],
                        lhsT=wq_m[m][:, cc, 0:M],
                        rhs=xsb_t[t][:, cc, :],
                        start=(cc == 0), stop=(cc == NCH - 1),
                    )
                nc.vector.tensor_scalar_add(
                    dest[:M, t * QC:(t + 1) * QC], ps[:M, :],
                    battn_sb[:M, m:m + 1],
                )

            def emit_vtrans(h, kt4):
                vsrc, vb = vt_of[h]
                pt = psB.tile([128, QC], F32, tag="psb", name="psb")
                for j in range(4):
                    kt = kt4 * 4 + j
                    nc.tensor.transpose(
                        pt[:, j * 64:(j + 1) * 64].bitcast(F32R),
                        vsrc[vb:vb + 64, kt * KT:(kt + 1) * KT],
                        ident[vb:vb + 64, vb:vb + 64],
                    )
                vi = h * NKT + kt4 * 4
                nc.vector.tensor_copy(
                    vaug[:, vi:vi + 4, 0:64],
                    pt[:, 0:256].rearrange("p (a b) -> p a b", b=64),
                )

            def emit_attn(h, t, drain):
                qt, kt_t, qb = qk_of[h]
                ydest, yrow = (yA, 0) if h == 0 else (yA, 64) if h == 1 else (yB, 0)
                qlo_g = t * QC
                py = psY.tile([128, QC], F32, tag="py", name="py")
                n_k = 4 * (t + 1)
                n_pair = n_k // 2

                def qlo_of(kt):
                    dm = kt - 4 * t
                    return 128 * dm if dm >= 0 else 0

                def emit_S(p):
                    # half 1's scores are written LEFT-SHIFTED by its qlo so
                    # the written psum range [lo0 : 1024-sh) is contiguous and
                    # one exp covers it exactly (no uninitialized hole).
                    lo0, sh = qlo_of(2 * p), qlo_of(2 * p + 1)
                    ps = psA.tile([128, 2 * QC], F32, tag="ps", name="ps")
                    pT = sbp.tile([128, 2 * QC], BF16, tag="pT", name="pT")
                    nc.tensor.matmul(
                        ps[:, lo0:QC],
                        lhsT=kt_t[qb:qb + 64, 2 * p * KT:(2 * p + 1) * KT],
                        rhs=qt[qb:qb + 64, qlo_g + lo0:qlo_g + QC],
                        start=True, stop=True,
                    )
                    nc.tensor.matmul(
                        ps[:, QC:2 * QC - sh],
                        lhsT=kt_t[qb:qb + 64, (2 * p + 1) * KT:(2 * p + 2) * KT],
                        rhs=qt[qb:qb + 64, qlo_g + sh:qlo_g + QC],
                        start=True, stop=True,
                    )
                    nc.scalar.activation(
                        pT[:, lo0:2 * QC - sh], ps[:, lo0:2 * QC - sh],
                        AF.Exp, scale=SCALE,
                    )
                    for half in range(2):
                        kt = 2 * p + half
                        if kt - 4 * t >= 0:
                            o = half * QC + (qlo_of(kt) if half == 0 else 0)
                            nc.vector.tensor_mul(
                                pT[:, o:o + 128], pT[:, o:o + 128],
                                trimask[:, :],
                            )
                    return pT

                def emit_PV(p, pT):
                    lo0, sh = qlo_of(2 * p), qlo_of(2 * p + 1)
                    nc.tensor.matmul(
                        py[0:65, lo0:QC],
                        lhsT=vaug[:, h * NKT + 2 * p, :],
                        rhs=pT[:, lo0:QC],
                        start=(2 * p == 0), stop=False,
                    )
                    nc.tensor.matmul(
                        py[0:65, sh:QC],
                        lhsT=vaug[:, h * NKT + 2 * p + 1, :],
                        rhs=pT[:, QC:2 * QC - sh],
                        start=False, stop=(2 * p + 1 == n_k - 1),
                    )

                pTs = {0: emit_S(0)}
                for p in range(n_pair):
                    if p + 1 < n_pair:
                        pTs[p + 1] = emit_S(p + 1)
                    emit_PV(p, pTs.pop(p))
                    drain(1)

                # fast approximate reciprocal of the denom row (psum in),
                # broadcast to 64 lanes on gpsimd, multiply on DVE.
                rec = sbp.tile([1, QC], F32, tag="rec", name="rec")
                den = sbp.tile([1, QC], F32, tag="den", name="den")
                nc.vector.tensor_copy(den[0:1, :], py[64:65, :])
                if USE_FAST_RECIP:
                    nc.vector.reciprocal_approx_fast(rec[0:1, :], den[0:1, :])
                else:
                    nc.vector.reciprocal(rec[0:1, :], den[0:1, :])
                bcast = sbp.tile([64, QC], F32, tag="bcast", name="bcast")
                nc.gpsimd.partition_broadcast(bcast[:, :], rec[0:1, :])
                nc.vector.tensor_mul(
                    ydest[yrow:yrow + 64, qlo_g:qlo_g + QC],
                    py[0:64, :], bcast[:, :],
                )

            osb_of = {}

            def emit_proj(ct, t):
                if t not in osb_of:
                    osb_of[t] = osbp.tile([128, NCH, QC], BF16, tag="osb",
                                          name="osb")
                osb = osb_of[t]
                ps = psB.tile([128, QC], F32, tag="psb", name="psb")
                nc.tensor.matmul(
                    ps[:, :],
                    lhsT=wproj_sb[:, 0, ct * 128:(ct + 1) * 128],
                    rhs=yA[:, t * QC:(t + 1) * QC],
                    start=True, stop=False,
                )
                nc.tensor.matmul(
                    ps[:, :],
                    lhsT=wproj_sb[0:64, 1, ct * 128:(ct + 1) * 128],
                    rhs=yB[0:64, t * QC:(t + 1) * QC],
                    start=False, stop=True,
                )
                # stage on DVE: the ACT queue is deep with exps during the
                # attention phase and would hold the psB tile for too long
                nc.vector.tensor_copy(osb[:, ct, :], ps[:, :])
                if ct % 2 == 1:
                    # issue the output DMA per ct-pair to overlap transfer
                    nc.sync.dma_start(
                        yTb[:, ct - 1:ct + 1, t * QC:(t + 1) * QC],
                        osb[:, ct - 1:ct + 1, :])

            # ---- work queue: everything not in prework, dependency order.
            WQ = []
            done = set()
            idx = [0]

            def push(tag, fn, *a):
                WQ.append((tag, fn, a))

            def drain(n):
                for _ in range(n):
                    if idx[0] >= len(WQ):
                        return
                    tag, fn, a = WQ[idx[0]]
                    idx[0] += 1
                    fn(*a)
                    done.add(tag)

            def ensure(*tags):
                want = set(tags) - done
                while want:
                    assert idx[0] < len(WQ), f"unsatisfiable prereqs {want}"
                    drain(1)
                    want -= done

            # t=0 extras
            push("qkv1_0", emit_qkv, 1, 0)
            push("vt1_0", emit_vtrans, 1, 0)
            push("vt2_0", emit_vtrans, 2, 0)
            push("qkv4_0", emit_qkv, 4, 0)
            for t in range(1, NQC):
                push(f"qkv0_{t}", emit_qkv, 0, t)
                push(f"qkv2_{t}", emit_qkv, 2, t)
                push(f"qkv3_{t}", emit_qkv, 3, t)
                push(f"vt0_{t}", emit_vtrans, 0, t)
                push(f"qkv1_{t}", emit_qkv, 1, t)
                push(f"vt1_{t}", emit_vtrans, 1, t)
                push(f"vt2_{t}", emit_vtrans, 2, t)
                push(f"qkv4_{t}", emit_qkv, 4, t)

            # ---- prework: get the first attention block running ASAP
            emit_qkv(0, 0)       # V01 chunk 0
            emit_vtrans(0, 0)
            nc.sync.dma_start(wq_m[2][:, :, :], wqkvb[:, 2, :, :])
            nc.sync.dma_start(wq_m[3][:, :, :], wqkvb[:, 3, :, :])
            emit_qkv(2, 0)       # Q01 chunk 0
            emit_qkv(3, 0)       # K01 chunk 0
            # bulk inputs last: nothing before this point waits on them
            nc.sync.dma_start(wq_m[1][:, :, :], wqkvb[:, 1, :, :])
            nc.sync.dma_start(wq_m[4][:, :, :], wqkvb[:, 4, :, :])
            for t in range(1, NQC):
                nc.sync.dma_start(xsb_t[t][:, :, :], xTb[:, t, :, :])
            nc.sync.dma_start(wproj_sb[:, :, :], wproj[:, :, :])

            prereq = [
                lambda t: [f"qkv2_{t}", f"qkv3_{t}", f"vt0_{t}"],
                lambda t: [f"qkv2_{t}", f"qkv3_{t}", f"vt1_{t}"],
                lambda t: [f"qkv1_{t}", f"qkv4_{t}", f"vt2_{t}"],
            ]
            done.update(["qkv0_0", "vt0_0", "qkv2_0", "qkv3_0"])

            for t in range(NQC):
                for h in range(HPC):
                    ensure(*[p for p in prereq[h](t) if p not in done])
                    emit_attn(h, t, drain)
                for ct in range(NCH):
                    push(f"proj_{ct}_{t}", emit_proj, ct, t)
            drain(len(WQ))

    nc.finalize()
    return nc


def kernel(x, W_attn, b_attn, W_proj, b_proj):
    global LAST_RESULTS
    B = x.shape[0]
    x = np.asarray(x, np.float32)
    W_attn = np.asarray(W_attn, np.float32)
    b_attn = np.asarray(b_attn, np.float32)
    W_proj = np.asarray(W_proj, np.float32)
    b_proj = np.asarray(b_proj, np.float32)

    if "nc" not in _CACHE:
        _CACHE["nc"] = build()
    nc = _CACHE["nc"]

    in_maps = []
    for c in range(8):
        b, g = divmod(c, 4)
        heads = [3 * g + i for i in range(HPC)]
        h0, h1, h2 = heads
        Q = lambda h: W_attn[:, 64 * h:64 * h + 64]
        K = lambda h: W_attn[:, C + 64 * h:C + 64 * h + 64]
        V = lambda h: W_attn[:, 2 * C + 64 * h:2 * C + 64 * h + 64]
        bQ = lambda h: b_attn[64 * h:64 * h + 64]
        bK = lambda h: b_attn[C + 64 * h:C + 64 * h + 64]
        bV = lambda h: b_attn[2 * C + 64 * h:2 * C + 64 * h + 64]
        # m-tiles: [V0|V1], [Q2|V2], [Q0|Q1], [K0|K1], [K2] — m-major 4D
        mt = [np.concatenate([V(h0), V(h1)], 1),
              np.concatenate([Q(h2), V(h2)], 1),
              np.concatenate([Q(h0), Q(h1)], 1),
              np.concatenate([K(h0), K(h1)], 1),
              np.concatenate([K(h2), np.zeros((C, 64), np.float32)], 1)]
        wqkvb = np.zeros((128, 5, NCH, 128), np.float32)
        for m, w in enumerate(mt):
            wqkvb[:, m] = w.reshape(NCH, 128, 128).transpose(1, 0, 2)
        wqkvb = np.ascontiguousarray(wqkvb).astype(ml_dtypes.bfloat16)
        bcols = [bV(h0), bV(h1), bQ(h2), bV(h2), bQ(h0), bQ(h1),
                 bK(h0), bK(h1), bK(h2), np.zeros(64, np.float32)]
        bvec = np.concatenate(bcols)                      # [640] = 5 x 128
        battn = np.ascontiguousarray(bvec.reshape(5, 128).T)  # [128, 5]
        wp = np.zeros((256, C), np.float32)
        wp[:192] = np.concatenate(
            [W_proj[64 * h:64 * h + 64, :] for h in heads], 0)
        wproj = np.ascontiguousarray(
            wp.reshape(2, 128, C).transpose(1, 0, 2)
        ).astype(ml_dtypes.bfloat16)                      # [128, 2, 768]
        # [128, NQC, NCH, QC]: xt[p, t, cc, q] = x^T[cc*128+p, t*QC+q]
        xt = np.ascontiguousarray(
            x[b].T.reshape(NCH, 128, NQC, QC).transpose(1, 2, 0, 3))
        in_maps.append({
            "xTb": xt.astype(ml_dtypes.bfloat16),
            "wqkvb": wqkvb,
            "battn": battn,
            "wproj": wproj,
        })

    res = run_bass_kernel_spmd(nc, in_maps, core_ids=list(range(8)))
    LAST_RESULTS = res

    out = np.zeros((B, T, C), np.float32)
    for c in range(8):
        b = c // 4
        yT = res.results[c]["yTb"].astype(np.float32)     # [128, 6, 2048]
        out[b] += yT.transpose(1, 0, 2).reshape(C, T).T
    out += b_proj
    return out


# revision 29
# speedup vs baseline: 1.0664x; 1.0013x over previous
"""Causal self-attention (B=2, T=2048, C=768, H=12) on 8 TRN2 NeuronCores.

Sharding: core c -> batch b = c//4, head-group g = c%4 (heads 3g..3g+2).
Each core computes QKV for its 3 heads, causal attention, and a partial
c_proj (its heads' rows of W_proj). Host sums the 4 partials per batch.

Device layout is fully transposed (feature dim on partitions):
  xT [128,6,2048], qkv^T tiles [128, 2048], scores S^T [k, q], y^T, out^T.
Softmax over k (= partition dim of S^T) uses an appended ones-column on V:
the PV matmul then yields [y_unnorm^T; denom] in one accumulation group.
No max-subtraction: scores are ~N(0,1) (|s| < ~7), exp is fp32-safe.

qkv m-tile packing (host must match); 5 tiles, no zero padding:
  m0: [V_h0 | V_h1]   m1: [Q_h2 | V_h2]   m2: [Q_h0 | Q_h1]
  m3: [K_h0 | K_h1]   m4: [K_h2] (64 wide)
Q_h and K_h of each head sit at the same base partition (matmul requires
equal lhsT/rhs base partitions): Q2 base 0 in m1, K2 base 0 in m4.

Schedule: a small prework block (V01+Q01+K01 chunk 0 + first V transpose)
starts the exp stream as early as possible; ALL remaining work (other QKV
chunks, V transposes, c_proj) is drained from a work queue one item per
attention pair, so the PE queue stays dense while ACT runs the exps (and
the PE p-state ramps to full clock). Normalize uses reciprocal_approx_fast
(single custom-DVE op, ~5x faster than InstReciprocal). DMAs are batched
(~600ns issue each on the sync sequencer, so fewer+bigger is critical).
Output partials are stored bf16 and DMA'd once per 512-wide q chunk.
"""

import numpy as np
import ml_dtypes

import concourse.bass as bass
import concourse.mybir as mybir
import concourse.tile as tile
from concourse import bacc
from concourse.bass_utils import run_bass_kernel_spmd
from concourse.masks import make_identity, make_upper_triangular

F32 = mybir.dt.float32
F32R = mybir.dt.float32r
BF16 = mybir.dt.bfloat16
AF = mybir.ActivationFunctionType

T = 2048           # sequence length
C = 768            # embed dim
HPC = 3            # heads per core
D = 64             # head dim
W576 = HPC * 3 * D  # 576 qkv cols per core
QC = 512           # q-chunk (psum bank width in fp32)
KT = 128           # k-tile
NKT = T // KT      # 16
NQC = T // QC      # 4
NCH = C // 128     # 6 contraction chunks for qkv
SCALE = 1.0 / 8.0  # 1/sqrt(64)
M_W = [128, 128, 128, 128, 64]
M_OFF = [0, 128, 256, 384, 512]
USE_FAST_RECIP = True

_CACHE = {}
LAST_RESULTS = None
_TCNT = [0]


def mk_persist(pool, shape, dtype, name=None):
    if name is None:
        _TCNT[0] += 1
        name = f"pt{_TCNT[0]}"
    return pool.tile(shape, dtype, name=name, tag=name)


def build():
    nc = bacc.Bacc("TRN2", target_bir_lowering=False)

    # x is t-chunk-major and wqkv m-tile-major so each prework DMA reads
    # one contiguous run per partition (128 fat descriptors, fast init)
    xTb = nc.dram_tensor("xTb", [128, NQC, NCH, QC], BF16, kind="ExternalInput")
    wqkvb = nc.dram_tensor("wqkvb", [128, 5, NCH, 128], BF16, kind="ExternalInput")
    battn = nc.dram_tensor("battn", [128, 5], F32, kind="ExternalInput")
    wproj = nc.dram_tensor("wproj", [128, 2, C], BF16, kind="ExternalInput")
    yTb = nc.dram_tensor("yTb", [128, NCH, T], BF16, kind="ExternalOutput")

    with tile.TileContext(nc) as tc, \
            tc.tile_pool(name="persist", bufs=1) as pp:
        # ---- persistent SBUF tensors ----
        ident_s = mk_persist(pp, [128, 128], F32)
        trimask_s = mk_persist(pp, [128, 128], F32)  # [k, q] = 1.0 iff k <= q
        make_identity(nc, ident_s[:, :])
        make_upper_triangular(nc, trimask_s[:, :], val=1.0, diag=True)
        ident = mk_persist(pp, [128, 128], F32R)
        trimask = mk_persist(pp, [128, 128], BF16)
        nc.vector.tensor_copy(ident[:, :], ident_s[:, :])
        nc.vector.tensor_copy(trimask[:, :], trimask_s[:, :])

        # input tiles split per DMA so readers only wait on their own region
        xsb_t = [mk_persist(pp, [128, NCH, QC], BF16) for _ in range(NQC)]
        wq_m = [mk_persist(pp, [128, NCH, 128], BF16) for _ in range(5)]
        battn_sb = mk_persist(pp, [128, 5], F32)
        wproj_sb = mk_persist(pp, [128, 2, C], BF16)  # [0]: rows 0:128, [1]: 128:192
        v01 = mk_persist(pp, [128, T], F32R)   # m0: [V_h0 | V_h1]
        q2v2 = mk_persist(pp, [128, T], F32R)  # m1: [Q_h2 | V_h2]
        qA = mk_persist(pp, [128, T], BF16)    # m2: [Q_h0 | Q_h1]
        kA = mk_persist(pp, [128, T], BF16)    # m3: [K_h0 | K_h1]
        k2 = mk_persist(pp, [64, T], F32R)     # m4: [K_h2]
        vaug = mk_persist(pp, [128, NKT * HPC, 65], BF16)  # V tiles + ones col
        yA = mk_persist(pp, [128, T], BF16)    # y^T heads 0,1
        yB = mk_persist(pp, [64, T], BF16)     # y^T head 2

        # The first matmul's DMA wait covers every dma_start emitted before
        # it, and transfers drain FIFO — so only the prework's inputs go
        # first (x chunk 0 split in halves so cc 0-2 land sooner);
        # everything else is emitted after the prework.
        nc.sync.dma_start(battn_sb[:, :], battn[:, :])
        nc.sync.dma_start(wq_m[0][:, :, :], wqkvb[:, 0, :, :])
        nc.sync.dma_start(xsb_t[0][:, 0:3, :], xTb[:, 0, 0:3, :])

        qkv_dest = [v01, q2v2, qA, kA, k2]
        # per head: (Q tile, K tile, base row); V^T source (tile, base row)
        qk_of = [(qA, kA, 0), (qA, kA, 64), (q2v2, k2, 0)]
        vt_of = [(v01, 0), (v01, 64), (q2v2, 64)]

        with (
            tc.tile_pool(name="psA", bufs=2, space="PSUM") as psA,
            tc.tile_pool(name="psB", bufs=2, space="PSUM") as psB,
            tc.tile_pool(name="psY", bufs=2, space="PSUM") as psY,
            tc.tile_pool(name="sb", bufs=8) as sbp,
            tc.tile_pool(name="osb", bufs=2) as osbp,
        ):
            # single strided memset for every vaug ones-column
            nc.vector.memset(vaug[:, :, 64:65], 1.0)

            def emit_qkv(m, t):
                M = M_W[m]
                dest = qkv_dest[m]
                ps = psB.tile([128, QC], F32, tag="psb", name="psb")
                for cc in range(NCH):
                    nc.tensor.matmul(
                        ps[:M, :],
                        lhsT=wq_m[m][:, cc, 0:M],
                        rhs=xsb_t[t][:, cc, :],
                        start=(cc == 0), stop=(cc == NCH - 1),
                    )
                nc.vector.tensor_scalar_add(
                    dest[:M, t * QC:(t + 1) * QC], ps[:M, :],
                    battn_sb[:M, m:m + 1],
                )

            def emit_vtrans(h, kt4):
                vsrc, vb = vt_of[h]
                pt = psB.tile([128, QC], F32, tag="psb", name="psb")
                for j in range(4):
                    kt = kt4 * 4 + j
                    nc.tensor.transpose(
                        pt[:, j * 64:(j + 1) * 64].bitcast(F32R),
                        vsrc[vb:vb + 64, kt * KT:(kt + 1) * KT],
                        ident[vb:vb + 64, vb:vb + 64],
                    )
                vi = h * NKT + kt4 * 4
                nc.vector.tensor_copy(
                    vaug[:, vi:vi + 4, 0:64],
                    pt[:, 0:256].rearrange("p (a b) -> p a b", b=64),
                )

            def emit_attn(h, t, drain):
                qt, kt_t, qb = qk_of[h]
                ydest, yrow = (yA, 0) if h == 0 else (yA, 64) if h == 1 else (yB, 0)
                qlo_g = t * QC
                py = psY.tile([128, QC], F32, tag="py", name="py")
                n_k = 4 * (t + 1)
                n_pair = n_k // 2

                def qlo_of(kt):
                    dm = kt - 4 * t
                    return 128 * dm if dm >= 0 else 0

                def emit_S(p):
                    # half 1's scores are written LEFT-SHIFTED by its qlo so
                    # the written psum range [lo0 : 1024-sh) is contiguous and
                    # one exp covers it exactly (no uninitialized hole).
                    lo0, sh = qlo_of(2 * p), qlo_of(2 * p + 1)
                    ps = psA.tile([128, 2 * QC], F32, tag="ps", name="ps")
                    pT = sbp.tile([128, 2 * QC], BF16, tag="pT", name="pT")
                    nc.tensor.matmul(
                        ps[:, lo0:QC],
                        lhsT=kt_t[qb:qb + 64, 2 * p * KT:(2 * p + 1) * KT],
                        rhs=qt[qb:qb + 64, qlo_g + lo0:qlo_g + QC],
                        start=True, stop=True,
                    )
                    nc.tensor.matmul(
                        ps[:, QC:2 * QC - sh],
                        lhsT=kt_t[qb:qb + 64, (2 * p + 1) * KT:(2 * p + 2) * KT],
                        rhs=qt[qb:qb + 64, qlo_g + sh:qlo_g + QC],
                        start=True, stop=True,
                    )
                    nc.scalar.activation(
                        pT[:, lo0:2 * QC - sh], ps[:, lo0:2 * QC - sh],
                        AF.Exp, scale=SCALE,
                    )
                    for half in range(2):
                        kt = 2 * p + half
                        if kt - 4 * t >= 0:
                            o = half * QC + (qlo_of(kt) if half == 0 else 0)
                            nc.vector.tensor_mul(
                                pT[:, o:o + 128], pT[:, o:o + 128],
                                trimask[:, :],
                            )
                    return pT

                def emit_PV(p, pT):
                    lo0, sh = qlo_of(2 * p), qlo_of(2 * p + 1)
                    nc.tensor.matmul(
                        py[0:65, lo0:QC],
                        lhsT=vaug[:, h * NKT + 2 * p, :],
                        rhs=pT[:, lo0:QC],
                        start=(2 * p == 0), stop=False,
                    )
                    nc.tensor.matmul(
                        py[0:65, sh:QC],
                        lhsT=vaug[:, h * NKT + 2 * p + 1, :],
                        rhs=pT[:, QC:2 * QC - sh],
                        start=False, stop=(2 * p + 1 == n_k - 1),
                    )

                pTs = {0: emit_S(0)}
                for p in range(n_pair):
                    if p + 1 < n_pair:
                        pTs[p + 1] = emit_S(p + 1)
                    emit_PV(p, pTs.pop(p))
                    drain(1)

                # fast approximate reciprocal of the denom row (psum in),
                # broadcast to 64 lanes on gpsimd, multiply on DVE.
                rec = sbp.tile([1, QC], F32, tag="rec", name="rec")
                den = sbp.tile([1, QC], F32, tag="den", name="den")
                nc.vector.tensor_copy(den[0:1, :], py[64:65, :])
                if USE_FAST_RECIP:
                    nc.vector.reciprocal_approx_fast(rec[0:1, :], den[0:1, :])
                else:
                    nc.vector.reciprocal(rec[0:1, :], den[0:1, :])
                bcast = sbp.tile([64, QC], F32, tag="bcast", name="bcast")
                nc.gpsimd.partition_broadcast(bcast[:, :], rec[0:1, :])
                nc.vector.tensor_mul(
                    ydest[yrow:yrow + 64, qlo_g:qlo_g + QC],
                    py[0:64, :], bcast[:, :],
                )

            osb_of = {}

            def emit_proj(ct, t):
                if t not in osb_of:
                    osb_of[t] = osbp.tile([128, NCH, QC], BF16, tag="osb",
                                          name="osb")
                osb = osb_of[t]
                ps = psB.tile([128, QC], F32, tag="psb", name="psb")
                nc.tensor.matmul(
                    ps[:, :],
                    lhsT=wproj_sb[:, 0, ct * 128:(ct + 1) * 128],
                    rhs=yA[:, t * QC:(t + 1) * QC],
                    start=True, stop=False,
                )
                nc.tensor.matmul(
                    ps[:, :],
                    lhsT=wproj_sb[0:64, 1, ct * 128:(ct + 1) * 128],
                    rhs=yB[0:64, t * QC:(t + 1) * QC],
                    start=False, stop=True,
                )
                # stage on DVE: the ACT queue is deep with exps during the
                # attention phase and would hold the psB tile for too long
                nc.vector.tensor_copy(osb[:, ct, :], ps[:, :])
                if ct % 2 == 1:
                    # issue the output DMA per ct-pair to overlap transfer
                    nc.sync.dma_start(
                        yTb[:, ct - 1:ct + 1, t * QC:(t + 1) * QC],
                        osb[:, ct - 1:ct + 1, :])

            # ---- work queue: everything not in prework, dependency order.
            WQ = []
            done = set()
            idx = [0]

            def push(tag, fn, *a):
                WQ.append((tag, fn, a))

            def drain(n):
                for _ in range(n):
                    if idx[0] >= len(WQ):
                        return
                    tag, fn, a = WQ[idx[0]]
                    idx[0] += 1
                    fn(*a)
                    done.add(tag)

            def ensure(*tags):
                want = set(tags) - done
                while want:
                    assert idx[0] < len(WQ), f"unsatisfiable prereqs {want}"
                    drain(1)
                    want -= done

            # t=0 extras
            push("qkv1_0", emit_qkv, 1, 0)
            push("vt1_0", emit_vtrans, 1, 0)
            push("vt2_0", emit_vtrans, 2, 0)
            push("qkv4_0", emit_qkv, 4, 0)
            for t in range(1, NQC):
                push(f"qkv0_{t}", emit_qkv, 0, t)
                push(f"qkv2_{t}", emit_qkv, 2, t)
                push(f"qkv3_{t}", emit_qkv, 3, t)
                push(f"vt0_{t}", emit_vtrans, 0, t)
                push(f"qkv1_{t}", emit_qkv, 1, t)
                push(f"vt1_{t}", emit_vtrans, 1, t)
                push(f"vt2_{t}", emit_vtrans, 2, t)
                push(f"qkv4_{t}", emit_qkv, 4, t)

            # ---- prework: get the first attention block running ASAP.
            # V01 chunk 0 emitted in cc-halves around the second x half-DMA
            # so its first matmuls wait only on the first 0.66MB of input.
            ps0 = psB.tile([128, QC], F32, tag="psb", name="psb")
            for cc in range(3):
                nc.tensor.matmul(
                    ps0[:, :], lhsT=wq_m[0][:, cc, :], rhs=xsb_t[0][:, cc, :],
                    start=(cc == 0), stop=False,
                )
            nc.sync.dma_start(xsb_t[0][:, 3:6, :], xTb[:, 0, 3:6, :])
            nc.sync.dma_start(wq_m[2][:, :, :], wqkvb[:, 2, :, :])
            for cc in range(3, NCH):
                nc.tensor.matmul(
                    ps0[:, :], lhsT=wq_m[0][:, cc, :], rhs=xsb_t[0][:, cc, :],
                    start=False, stop=(cc == NCH - 1),
                )
            nc.vector.tensor_scalar_add(
                v01[:, 0:QC], ps0[:, :], battn_sb[:, 0:1])
            emit_vtrans(0, 0)
            nc.sync.dma_start(wq_m[3][:, :, :], wqkvb[:, 3, :, :])
            emit_qkv(2, 0)       # Q01 chunk 0
            emit_qkv(3, 0)       # K01 chunk 0
            # bulk inputs last: nothing before this point waits on them
            nc.sync.dma_start(wq_m[1][:, :, :], wqkvb[:, 1, :, :])
            nc.sync.dma_start(wq_m[4][:, :, :], wqkvb[:, 4, :, :])
            for t in range(1, NQC):
                nc.sync.dma_start(xsb_t[t][:, :, :], xTb[:, t, :, :])
            nc.sync.dma_start(wproj_sb[:, :, :], wproj[:, :, :])

            prereq = [
                lambda t: [f"qkv2_{t}", f"qkv3_{t}", f"vt0_{t}"],
                lambda t: [f"qkv2_{t}", f"qkv3_{t}", f"vt1_{t}"],
                lambda t: [f"qkv1_{t}", f"qkv4_{t}", f"vt2_{t}"],
            ]
            done.update(["qkv0_0", "vt0_0", "qkv2_0", "qkv3_0"])

            for t in range(NQC):
                for h in range(HPC):
                    ensure(*[p for p in prereq[h](t) if p not in done])
                    emit_attn(h, t, drain)
                for ct in range(NCH):
                    push(f"proj_{ct}_{t}", emit_proj, ct, t)
            drain(len(WQ))

    nc.finalize()
    return nc


def kernel(x, W_attn, b_attn, W_proj, b_proj):
    global LAST_RESULTS
    B = x.shape[0]
    x = np.asarray(x, np.float32)
    W_attn = np.asarray(W_attn, np.float32)
    b_attn = np.asarray(b_attn, np.float32)
    W_proj = np.asarray(W_proj, np.float32)
    b_proj = np.asarray(b_proj, np.float32)

    if "nc" not in _CACHE:
        _CACHE["nc"] = build()
    nc = _CACHE["nc"]

    in_maps = []
    for c in range(8):
        b, g = divmod(c, 4)
        heads = [3 * g + i for i in range(HPC)]
        h0, h1, h2 = heads
        Q = lambda h: W_attn[:, 64 * h:64 * h + 64]
        K = lambda h: W_attn[:, C + 64 * h:C + 64 * h + 64]
        V = lambda h: W_attn[:, 2 * C + 64 * h:2 * C + 64 * h + 64]
        bQ = lambda h: b_attn[64 * h:64 * h + 64]
        bK = lambda h: b_attn[C + 64 * h:C + 64 * h + 64]
        bV = lambda h: b_attn[2 * C + 64 * h:2 * C + 64 * h + 64]
        # m-tiles: [V0|V1], [Q2|V2], [Q0|Q1], [K0|K1], [K2] — m-major 4D
        mt = [np.concatenate([V(h0), V(h1)], 1),
              np.concatenate([Q(h2), V(h2)], 1),
              np.concatenate([Q(h0), Q(h1)], 1),
              np.concatenate([K(h0), K(h1)], 1),
              np.concatenate([K(h2), np.zeros((C, 64), np.float32)], 1)]
        wqkvb = np.zeros((128, 5, NCH, 128), np.float32)
        for m, w in enumerate(mt):
            wqkvb[:, m] = w.reshape(NCH, 128, 128).transpose(1, 0, 2)
        wqkvb = np.ascontiguousarray(wqkvb).astype(ml_dtypes.bfloat16)
        bcols = [bV(h0), bV(h1), bQ(h2), bV(h2), bQ(h0), bQ(h1),
                 bK(h0), bK(h1), bK(h2), np.zeros(64, np.float32)]
        bvec = np.concatenate(bcols)                      # [640] = 5 x 128
        battn = np.ascontiguousarray(bvec.reshape(5, 128).T)  # [128, 5]
        wp = np.zeros((256, C), np.float32)
        wp[:192] = np.concatenate(
            [W_proj[64 * h:64 * h + 64, :] for h in heads], 0)
        wproj = np.ascontiguousarray(
            wp.reshape(2, 128, C).transpose(1, 0, 2)
        ).astype(ml_dtypes.bfloat16)                      # [128, 2, 768]
        # [128, NQC, NCH, QC]: xt[p, t, cc, q] = x^T[cc*128+p, t*QC+q]
        xt = np.ascontiguousarray(
            x[b].T.reshape(NCH, 128, NQC, QC).transpose(1, 2, 0, 3))
        in_maps.append({
            "xTb": xt.astype(ml_dtypes.bfloat16),
            "wqkvb": wqkvb,
            "battn": battn,
            "wproj": wproj,
        })

    res = run_bass_kernel_spmd(nc, in_maps, core_ids=list(range(8)))
    LAST_RESULTS = res

    out = np.zeros((B, T, C), np.float32)
    for c in range(8):
        b = c // 4
        yT = res.results[c]["yTb"].astype(np.float32)     # [128, 6, 2048]
        out[b] += yT.transpose(1, 0, 2).reshape(C, T).T
    out += b_proj
    return out
